# revision 28
# baseline (speedup 1.0000x reference)
"""Trainium2 Bass kernel for nn_MixedLipMlp (soft-MoE MLP with Lipschitz gate).

Strategy: data-parallel over batch B=4096 across 8 NeuronCores (512 rows each,
expert weights + gate replicated). Everything on-chip is computed feature-major
(features on partitions, batch on the free dim) so no transposes are needed
anywhere: activations stream as matmul rhs, weights as lhsT.

Per-core compute:
  gate:   hT = elu(scale ⊙ (gwT.T @ xT) + gb)  (scale = lip row scaling, applied
          per-partition post-matmul), softmax over the 8 expert logits done with
          an exp (ACT) + ones-matmul column sum (PE) + reciprocal (DVE).
  moe:    out = Σ_e coeff_e ⊙ (x @ W_e) + coeff @ b is restructured as
          Σ_e (coeff_e ⊙ x) @ W_e  — the per-expert coefficient scaling moves to
          the *inputs*, so all E×K_tiles matmuls accumulate into a single PSUM
          bank per output m-tile, and the bias term folds in as one extra K=8
          matmul with rhs=coeffT. coeff_e rows are broadcast across partitions
          with K=1 ones-matmuls on the PE.
  layer2: output dim is only 12, so the coefficient mix is applied on the
          *outputs* (per-expert PSUM tile, then 2 small DVE ops per expert)
          which avoids 8 scaled copies of the 576-row input.
  elu(y) = min(exp(y)-1, relu(y)) — 2 ACT + 1 fused DVE op per tile, exact and
          overflow-safe.

Scheduling: weights are host-packed into one contiguous DMA per expert per
layer (z-slabs for all experts fused into one tensor), and each MoE layer runs
its z-feature matmuls for all 8 experts first — that work depends only on the
coefficients, so the PE stays busy while the previous layer's ELU outputs are
being scaled per-expert.
"""

import os
import sys

if "/opt/trn_rl_repo" not in sys.path:
    sys.path.insert(0, "/opt/trn_rl_repo")

# recover cleanly if a previous process left the NeuronCores wedged
os.environ.setdefault("NEURON_RT_RESET_CORES", "1")

import numpy as np

# Problem dimensions (hardcoded; must match the grader's setup_inputs()).
B = 4096
NCORES = 8
BS = B // NCORES  # 512 batch rows per core = matmul free dim
LATENT = 64
INPUT_SIZE = 256
IN_DIM = LATENT + INPUT_SIZE  # 320
HIDDEN = 512
ACTIONS = 12
E = 8
GATE_H = 128
INTER = HIDDEN + LATENT  # 576

# h-feature k-slab counts per layer (after the 64-row z slab)
NK0 = 2  # layer0: c has 256 rows = 2 slabs
NK12 = 4  # layers1,2: h has 512 rows = 4 slabs

# matmul input dtype knobs: float32 (exact, 4 cyc/row) vs float32r (1 cyc/row
# at N>=512 but ~1.5e-4 relative multiply error on HW).
MOE_F32R = True
GATE_F32R = False

TRACE = False
LAST_EXEC_NS = None
LAST_RESULTS = None


def _build_nc():
    import concourse.mybir as mybir
    from concourse import bacc
    from concourse.tile import TileContext

    dt = mybir.dt
    F32 = dt.float32
    F32R = dt.float32r
    AF = mybir.ActivationFunctionType
    OP = mybir.AluOpType

    WDT = F32R if MOE_F32R else F32

    nc = bacc.Bacc("TRN2", target_bir_lowering=False)

    # ---- DRAM I/O (weights host-packed into lhsT slab layouts) ----------
    d_zT = nc.dram_tensor("zT", [LATENT, BS], F32, kind="ExternalInput")
    d_cT = nc.dram_tensor("cT", [INPUT_SIZE, BS], F32, kind="ExternalInput")
    # z-slabs packed in expert pairs: rows 0:64 = even expert, 64:128 = odd
    d_w0z = nc.dram_tensor("w0z", [128, E // 2 * HIDDEN], WDT, kind="ExternalInput")
    d_w1z = nc.dram_tensor("w1z", [128, E // 2 * HIDDEN], WDT, kind="ExternalInput")
    d_w2z = nc.dram_tensor("w2z", [128, E // 2 * 32], WDT, kind="ExternalInput")
    # h-slabs per expert: (E, 128, nk*out)
    d_w0h = nc.dram_tensor("w0h", [E, 128, NK0 * HIDDEN], WDT, kind="ExternalInput")
    d_w1h = nc.dram_tensor("w1h", [E, 128, NK12 * HIDDEN], WDT, kind="ExternalInput")
    d_w2h = nc.dram_tensor("w2h", [E, 128, NK12 * 32], WDT, kind="ExternalInput")
    d_b0 = nc.dram_tensor("b0", [E, HIDDEN], F32, kind="ExternalInput")
    d_b1 = nc.dram_tensor("b1", [E, HIDDEN], F32, kind="ExternalInput")
    d_b2 = nc.dram_tensor("b2", [E, ACTIONS], F32, kind="ExternalInput")
    d_gwT = [
        nc.dram_tensor("gw0T", [IN_DIM, GATE_H], F32, kind="ExternalInput"),
        nc.dram_tensor("gw1T", [GATE_H, GATE_H], F32, kind="ExternalInput"),
        nc.dram_tensor("gw2T", [GATE_H, E], F32, kind="ExternalInput"),
    ]
    d_gb = [
        nc.dram_tensor("gb0", [GATE_H, 1], F32, kind="ExternalInput"),
        nc.dram_tensor("gb1", [GATE_H, 1], F32, kind="ExternalInput"),
        nc.dram_tensor("gb2", [E, 1], F32, kind="ExternalInput"),
    ]
    d_gc = [
        nc.dram_tensor("gc0", [1, 1], F32, kind="ExternalInput"),
        nc.dram_tensor("gc1", [1, 1], F32, kind="ExternalInput"),
        nc.dram_tensor("gc2", [1, 1], F32, kind="ExternalInput"),
    ]
    d_zTr = nc.dram_tensor("zTr", [LATENT, BS], WDT, kind="ExternalInput")
    d_out = nc.dram_tensor("outT", [ACTIONS, BS], F32, kind="ExternalOutput")

    def mm(ps, lhsT, rhs, start, stop, f32r=False):
        # operand tile dtypes (float32 vs float32r) select the PE mode
        nc.tensor.matmul(ps, lhsT, rhs, start=start, stop=stop)

    with TileContext(nc) as tc:
        from contextlib import ExitStack

        with ExitStack() as ctx:
            pers = ctx.enter_context(tc.tile_pool(name="pers", bufs=1))
            wstr = ctx.enter_context(tc.tile_pool(name="wstr", bufs=3))
            sca = ctx.enter_context(tc.tile_pool(name="sca", bufs=10))
            etmp = ctx.enter_context(tc.tile_pool(name="etmp", bufs=3))
            gtmp = ctx.enter_context(tc.tile_pool(name="gtmp", bufs=2))

            # ---- constants, gate weights first (they gate the PE start) --
            ones_col = pers.tile([128, 1], F32, tag="ones_col")
            nc.vector.memset(ones_col, 1.0)
            ones_blk = pers.tile([128, 128], F32, tag="ones_blk")
            nc.vector.memset(ones_blk, 1.0)
            ones_row = ones_blk[0:1, :]

            gw0t = []
            for i in range(3):
                k0, k1 = [(0, 64), (64, 192), (192, 320)][i]
                t = pers.tile([k1 - k0, GATE_H], F32, tag=f"gw0_{i}")
                nc.sync.dma_start(out=t, in_=d_gwT[0][k0:k1, :])
                gw0t.append(t)
            gw1t = pers.tile([GATE_H, GATE_H], F32, tag="gw1")
            nc.sync.dma_start(out=gw1t, in_=d_gwT[1][:, :])
            gw2t = pers.tile([GATE_H, E], F32, tag="gw2")
            nc.sync.dma_start(out=gw2t, in_=d_gwT[2][:, :])

            gdims = [GATE_H, GATE_H, E]
            gbias, lipc = [], []
            for i, mg in enumerate(gdims):
                t = pers.tile([mg, 1], F32, tag=f"gb{i}")
                nc.sync.dma_start(out=t, in_=d_gb[i][:, :])
                gbias.append(t)
                raw = pers.tile([mg, 1], F32, tag=f"gcraw{i}")
                nc.gpsimd.dma_start(
                    out=raw, in_=d_gc[i][:, :].to_broadcast([mg, 1])
                )
                # softplus(x) = ln(exp(x) + 1); Softplus has no ACT table here
                t1 = pers.tile([mg, 1], F32, tag=f"lipce{i}")
                nc.scalar.activation(out=t1, in_=raw, func=AF.Exp)
                t2 = pers.tile([mg, 1], F32, tag=f"lipc{i}")
                nc.scalar.activation(out=t2, in_=t1, func=AF.Ln, bias=1.0)
                lipc.append(t2)

            xz = pers.tile([LATENT, BS], F32, tag="xz")
            nc.sync.dma_start(out=xz, in_=d_zT[:, :])
            xc = []
            for i in range(2):
                t = pers.tile([128, BS], F32, tag=f"xc{i}")
                nc.sync.dma_start(out=t, in_=d_cT[128 * i : 128 * (i + 1), :])
                xc.append(t)

            b0sb = pers.tile([E, HIDDEN], F32, tag="b0sb")
            nc.sync.dma_start(out=b0sb, in_=d_b0[:, :])
            b1sb = pers.tile([E, HIDDEN], F32, tag="b1sb")
            nc.sync.dma_start(out=b1sb, in_=d_b1[:, :])
            b2sb = pers.tile([E, ACTIONS], F32, tag="b2sb")
            nc.sync.dma_start(out=b2sb, in_=d_b2[:, :])

            # z-slab weights (small, persistent) + the l0 h-slab stream
            w0z = pers.tile([128, E // 2 * HIDDEN], WDT, tag="w0z")
            nc.sync.dma_start(out=w0z, in_=d_w0z[:, :])
            w1z = pers.tile([128, E // 2 * HIDDEN], WDT, tag="w1z")
            nc.sync.dma_start(out=w1z, in_=d_w1z[:, :])
            w2z = pers.tile([128, E // 2 * 32], WDT, tag="w2z")
            nc.sync.dma_start(out=w2z, in_=d_w2z[:, :])
            # zT duplicated into both 64-row halves for row-paired matmuls
            xz2 = pers.tile([128, BS], WDT, tag="xz2")
            nc.sync.dma_start(out=xz2[:LATENT, :], in_=d_zTr[:, :])
            nc.sync.dma_start(out=xz2[LATENT:, :], in_=d_zTr[:, :])

            w0h = []
            for e in range(E):
                t = wstr.tile([128, NK0 * HIDDEN], WDT, tag="w0h", name=f"w0h{e}")
                eng = nc.sync if e % 2 == 0 else nc.scalar
                eng.dma_start(out=t, in_=d_w0h[e, :, :])
                w0h.append(t)

            def elu_from_psum(ps, scale, bias, out_tag, odt=F32):
                ex = etmp.tile([ps.shape[0], BS], F32, tag="elu_exp")
                nc.scalar.activation(
                    out=ex, in_=ps, func=AF.Exp, bias=bias, scale=scale
                )
                rl = etmp.tile([ps.shape[0], BS], F32, tag="elu_relu")
                nc.scalar.activation(
                    out=rl, in_=ps, func=AF.Relu, bias=bias, scale=scale
                )
                h = pers.tile([ps.shape[0], BS], odt, tag=out_tag)
                # elu = min(exp(y)-1, relu(y))
                nc.vector.scalar_tensor_tensor(
                    out=h, in0=ex, scalar=1.0, in1=rl,
                    op0=OP.subtract, op1=OP.min,
                )
                return h

            # ---- lip row scales: scale = min(softplus(gc)/rowsum|W|, 1) --
            gscale = []
            with tc.tile_pool(name="ps_misc", bufs=1, space="PSUM") as ps_misc:
                # HAM warm-up: the gate phase is serial and sparse on the PE,
                # which keeps the clock gate at 4/8; a burst of throwaway
                # matmuls trips the activity monitor to full speed
                def filler(n, rhs, nfree):
                    for i_ in range(n):
                        pw = ps_misc.tile([128, nfree], F32, tag="warm",
                                          name=f"warm{nc.next_id()}")
                        nc.tensor.matmul(pw, ones_blk, rhs, start=True,
                                         stop=True)

                warm_rhs = gtmp.tile([128, BS], F32, tag="warm_rhs", bufs=1)
                nc.vector.memset(warm_rhs, 0.0)
                filler(12, warm_rhs, BS)
                for i, mg in enumerate(gdims):
                    wt = [gw0t, [gw1t], [gw2t]][i]
                    ps_rs = ps_misc.tile([mg, 1], F32, tag="rs")
                    for k, t in enumerate(wt):
                        a = gtmp.tile([t.shape[0], mg], F32, tag="gabs")
                        nc.scalar.activation(out=a, in_=t, func=AF.Abs)
                        mm(ps_rs, a, ones_col[: t.shape[0], :],
                           start=(k == 0), stop=(k == len(wt) - 1), f32r=False)
                    rec = gtmp.tile([mg, 1], F32, tag="grec")
                    nc.vector.reciprocal(out=rec, in_=ps_rs)
                    sc = pers.tile([mg, 1], F32, tag=f"gscale{i}")
                    nc.vector.tensor_scalar(
                        out=sc, in0=rec, scalar1=lipc[i], scalar2=1.0,
                        op0=OP.mult, op1=OP.min,
                    )
                    gscale.append(sc)

                # ---- gate forward (feature-major) ------------------------
                ps_g0 = ps_misc.tile([GATE_H, BS], F32, tag="g", name="psg0")
                rhs0 = [xz, xc[0], xc[1]]
                for k in range(3):
                    mm(ps_g0, gw0t[k], rhs0[k], start=(k == 0), stop=(k == 2),
                       f32r=GATE_F32R)
                h0g = elu_from_psum(ps_g0, gscale[0], gbias[0], "h0g")
                filler(4, xc[0], BS)

                ps_g1 = ps_misc.tile([GATE_H, BS], F32, tag="g", name="psg1")
                mm(ps_g1, gw1t, h0g, start=True, stop=True, f32r=GATE_F32R)
                h1g = elu_from_psum(ps_g1, gscale[1], gbias[1], "h1g")
                filler(4, xc[0], BS)

                ps_lg = ps_misc.tile([E, BS], F32, tag="lg")
                mm(ps_lg, gw2t, h1g, start=True, stop=True, f32r=GATE_F32R)
                filler(4, xc[0], BS)

                # softmax over the 8 expert partitions (logits are bounded,
                # so no max subtraction): expl = exp(scale*logits + gb)
                expl = pers.tile([E, BS], F32, tag="expl")
                nc.scalar.activation(
                    out=expl, in_=ps_lg, func=AF.Exp,
                    bias=gbias[2], scale=gscale[2],
                )
                ps_sum = ps_misc.tile([1, BS], F32, tag="sum")
                mm(ps_sum, ones_col[:E, :], expl, start=True, stop=True, f32r=False)
                recip = pers.tile([1, BS], F32, tag="recip")
                nc.vector.reciprocal(out=recip, in_=ps_sum)

            # broadcast coefficient rows to all 128 partitions with K=1
            # ones-matmuls; normalization by 1/sum fused into the PSUM->SBUF
            # multiply
            with tc.tile_pool(name="ps_bc", bufs=4, space="PSUM") as ps_bc:
                ps_bcr = ps_bc.tile([128, BS], F32, tag="bc")
                mm(ps_bcr, ones_row, recip, start=True, stop=True, f32r=False)
                bcR = pers.tile([128, BS], F32, tag="bcR")
                nc.vector.tensor_copy(out=bcR, in_=ps_bcr)

                coeffT = pers.tile([E, BS], F32, tag="coeffT")
                nc.vector.tensor_mul(coeffT, expl, bcR[:E, :])

                # stage the 8 exp rows at partition offsets 0/32/64/96 of two
                # tiles, then run the K=1 broadcasts 4-at-a-time in distinct
                # PE row groups draining to distinct PSUM banks
                rows4 = []
                for g in range(2):
                    rt = etmp.tile([128, BS], F32, tag="bcrow", name=f"rows4_{g}")
                    for j in range(4):
                        nc.sync.dma_start(
                            out=rt[32 * j : 32 * j + 1, :],
                            in_=expl[4 * g + j : 4 * g + j + 1, :],
                        )
                    rows4.append(rt)
                bcE = []
                for e in range(E):
                    g, j = divmod(e, 4)
                    pb = ps_bc.tile([128, BS], F32, tag="bc", name=f"pbc{e}")
                    nc.tensor.matmul(
                        pb, ones_blk[32 * j : 32 * j + 1, :],
                        rows4[g][32 * j : 32 * j + 1, :],
                        start=True, stop=True, tile_position=(32 * j, 0),
                    )
                    t = pers.tile([128, BS], F32, tag=f"bcE{e}")
                    nc.vector.tensor_mul(t, pb, bcR)
                    bcE.append(t)

            # coeff-scaled z per expert, duplicated into both 64-row halves
            # so the expert can ride either PE row group; shared by l0 and l1
            zsf = []
            for e in range(E):
                t = pers.tile([128, BS], WDT, tag=f"zsf{e}")
                nc.vector.tensor_mul(
                    t[:LATENT, :], xz2[:LATENT, :], bcE[e][:LATENT, :]
                )
                nc.vector.tensor_mul(
                    t[LATENT:, :], xz2[LATENT:, :], bcE[e][LATENT:, :]
                )
                zsf.append(t)

            n_m = HIDDEN // 128  # 4

            def moe_layer(wz, wh, hs_src, nk, bsb, psl, f32r):
                # pass A: row-paired z matmuls — two experts concurrently in
                # disjoint PE row groups. The top group only ever drains to
                # banks {0,1} and the bottom to {2,3} (concurrent row groups
                # writing one bank is a hardware hazard); the T1/T2 packing
                # swaps experts between groups so each covers all 4 m-slices.
                for p in range(E // 2):
                    for t_ in range(2):
                        base = p * HIDDEN + t_ * 256
                        etop = 2 * p + t_
                        ebot = 2 * p + 1 - t_
                        st = p == 0 and t_ == 0
                        for mi in range(2):
                            mm(psl[mi],
                               wz[:LATENT, base + 128 * mi : base + 128 * (mi + 1)],
                               zsf[etop][:LATENT, :],
                               start=st, stop=False, f32r=f32r)
                            mm(psl[2 + mi],
                               wz[LATENT:, base + 128 * mi : base + 128 * (mi + 1)],
                               zsf[ebot][LATENT:, :],
                               start=st, stop=False, f32r=f32r)
                # pass B: per-expert scaled h inputs + their matmuls
                for e in range(E):
                    hs = []
                    for i in range(nk):
                        t = sca.tile([128, BS], WDT, tag="s", name=f"s{e}_{i}")
                        nc.vector.tensor_mul(t, hs_src[i], bcE[e])
                        hs.append(t)
                    for ki in range(nk):
                        for m in range(n_m):
                            mm(psl[m], wh[e][:, ki * HIDDEN + 128 * m :
                                             ki * HIDDEN + 128 * (m + 1)],
                               hs[ki], start=False, stop=False, f32r=f32r)
                # bias: out += coeff @ b  (K=8 matmul closes each bank)
                for m in range(n_m):
                    mm(psl[m], bsb[:, 128 * m : 128 * (m + 1)], coeffT,
                       start=False, stop=True, f32r=f32r)

            # ---- MoE layers 0+1 share all 8 PSUM banks so layer 1's
            # z-pass can start while layer 0's ELU epilogue drains ---------
            acc_ctx = tc.tile_pool(name="ps_acc", bufs=8, space="PSUM")
            ps_acc = acc_ctx.__enter__()
            ps_l0 = [ps_acc.tile([128, BS], F32, tag="acc", name=f"psl0_{m}")
                     for m in range(n_m)]
            moe_layer(w0z, w0h, xc, NK0, b0sb, ps_l0, MOE_F32R)
            h0m = [elu_from_psum(ps_l0[m], 1.0, 0.0, f"h0m{m}")
                   for m in range(n_m)]

            # l1 h-slab weight stream (DMA follows l0's in queue order)
            w1h = []
            for e in range(E):
                t = wstr.tile([128, NK12 * HIDDEN], WDT, tag="w1h", name=f"w1h{e}")
                eng = nc.sync if e % 2 == 0 else nc.scalar
                eng.dma_start(out=t, in_=d_w1h[e, :, :])
                w1h.append(t)

            # ---- MoE layer 1: (576 -> 512), elu ------------------------
            ps_l1 = [ps_acc.tile([128, BS], F32, tag="acc", name=f"psl1_{m}")
                     for m in range(n_m)]
            moe_layer(w1z, w1h, h0m, NK12, b1sb, ps_l1, MOE_F32R)
            h1m = [elu_from_psum(ps_l1[m], 1.0, 0.0, f"h1m{m}", odt=WDT)
                   for m in range(n_m)]
            acc_ctx.__exit__(None, None, None)

            # ---- MoE layer 2: (576 -> 12), coefficient mix on outputs --
            w2h = []
            for e in range(E):
                t = wstr.tile([128, NK12 * 32], WDT, tag="w2h", name=f"w2h{e}", bufs=8)
                nc.gpsimd.dma_start(out=t, in_=d_w2h[e, :, :])
                w2h.append(t)

            # per-expert f32r chains (f32r matmuls reject column-group
            # tile_position, so experts run sequentially into their own banks)
            out_acc = pers.tile([ACTIONS, BS], F32, tag="out_acc")
            with tc.tile_pool(name="ps_l2", bufs=3, space="PSUM") as ps_l2:
                pbias = ps_l2.tile([ACTIONS, BS], F32, tag="l2b", bufs=1)
                mm(pbias, b2sb, coeffT, start=True, stop=True)
                acc2 = pers.tile([ACTIONS, BS], F32, tag="acc2")
                nc.vector.tensor_copy(out=acc2, in_=pbias)
                first = [None, acc2]
                for e in range(E):
                    pe_ = ps_l2.tile([32, BS], F32, tag="l2", name=f"pl2{e}")
                    par = e % 2
                    mm(pe_, w2z[64 * par : 64 * par + LATENT,
                                (e // 2) * 32 : (e // 2 + 1) * 32],
                       xz2[64 * par : 64 * par + LATENT, :],
                       start=True, stop=False)
                    for ki in range(NK12):
                        mm(pe_, w2h[e][:, ki * 32 : (ki + 1) * 32], h1m[ki],
                           start=False, stop=(ki == NK12 - 1))
                    t = sca.tile([ACTIONS, BS], F32, tag="mx", name=f"mx{e}")
                    nc.vector.tensor_mul(t, pe_[:ACTIONS, :], bcE[e][:ACTIONS, :])
                    c_ = e % 2
                    if first[c_] is None:
                        first[c_] = t
                    else:
                        nc.vector.tensor_add(first[c_], first[c_], t)
                nc.vector.tensor_add(out_acc, first[0], first[1])

            nc.sync.dma_start(out=d_out[:, :], in_=out_acc)

    nc.finalize()
    return nc


_nc_cache = None


def _get_nc():
    global _nc_cache
    if _nc_cache is None:
        _nc_cache = _build_nc()
    return _nc_cache


def _patch_hook_errors():
    # exceptions inside the neuronx-cc hook are swallowed by the PJRT
    # plugin ("CallFunctionObjArgs: error condition"); print them here
    from concourse import bass2jax

    orig = bass2jax.neuronx_cc_hook
    if getattr(orig, "_err_patched", False):
        return

    def wrapped(*a, **k):
        import traceback

        try:
            return orig(*a, **k)
        except BaseException as e:
            print(getattr(e, "output", ""), file=sys.stderr)
            traceback.print_exc()
            raise

    wrapped._err_patched = True
    bass2jax.neuronx_cc_hook = wrapped


def _pack_z_pairs(w):
    # (E, in, out) -> (128, E/2*out). For each expert pair p, two tiles of
    # (128, out/2): T1 = [top: even expert, first half of m-slices;
    # bottom: odd expert, second half], T2 = the swap — so the top PE row
    # group only ever produces the first half of output banks and the bottom
    # the second half, while both experts cover all output columns.
    z = w[:, :LATENT, :]
    out = z.shape[2]
    h = out // 2
    blk = np.empty((128, E // 2, 2, h), np.float32)
    for p in range(E // 2):
        blk[:LATENT, p, 0] = z[2 * p, :, :h]
        blk[LATENT:, p, 0] = z[2 * p + 1, :, h:]
        blk[:LATENT, p, 1] = z[2 * p + 1, :, :h]
        blk[LATENT:, p, 1] = z[2 * p, :, h:]
    return np.ascontiguousarray(blk.reshape(128, -1))


def _pad_w2(w):
    # pad the 12-wide output dim to 32 (f32r matmuls reject M=12; zero rows
    # also make every PSUM partition in the col-tiled banks defined)
    out = np.zeros((E, INTER, 32), np.float32)
    out[:, :, :ACTIONS] = w
    return out


def _pack_z_simple(w):
    # (E, in, out) -> (128, E/2*out): rows 0:64 = even expert z-slab,
    # rows 64:128 = odd expert (layer2 drains each expert to its own bank)
    z = w[:, :LATENT, :]
    top = z[0::2].transpose(1, 0, 2).reshape(LATENT, -1)
    bot = z[1::2].transpose(1, 0, 2).reshape(LATENT, -1)
    return np.ascontiguousarray(np.concatenate([top, bot], axis=0))


def _pack_weights(f):
    c = np.ascontiguousarray
    return {
        "w0z": _pack_z_pairs(f["w0"]),
        "w1z": _pack_z_pairs(f["w1"]),
        "w2z": _pack_z_simple(_pad_w2(f["w2"])),
        "w0h": c(f["w0"][:, LATENT:, :].reshape(E, NK0, 128, HIDDEN)
                 .transpose(0, 2, 1, 3).reshape(E, 128, NK0 * HIDDEN)),
        "w1h": c(f["w1"][:, LATENT:, :].reshape(E, NK12, 128, HIDDEN)
                 .transpose(0, 2, 1, 3).reshape(E, 128, NK12 * HIDDEN)),
        "w2h": c(_pad_w2(f["w2"])[:, LATENT:, :].reshape(E, NK12, 128, 32)
                 .transpose(0, 2, 1, 3).reshape(E, 128, NK12 * 32)),
    }


def kernel(**inputs):
    global LAST_EXEC_NS, LAST_RESULTS
    from concourse import bass_utils

    _patch_hook_errors()

    f = {k: np.ascontiguousarray(np.asarray(v, dtype=np.float32))
         for k, v in inputs.items()}

    shared = _pack_weights(f)
    shared.update({
        "b0": f["b0"], "b1": f["b1"], "b2": f["b2"],
        "gw0T": np.ascontiguousarray(f["gw0"].T),
        "gw1T": np.ascontiguousarray(f["gw1"].T),
        "gw2T": np.ascontiguousarray(f["gw2"].T),
        "gb0": f["gb0"].reshape(GATE_H, 1),
        "gb1": f["gb1"].reshape(GATE_H, 1),
        "gb2": f["gb2"].reshape(E, 1),
        "gc0": f["gc0"].reshape(1, 1),
        "gc1": f["gc1"].reshape(1, 1),
        "gc2": f["gc2"].reshape(1, 1),
    })
    in_maps = []
    for c in range(NCORES):
        sl = slice(c * BS, (c + 1) * BS)
        m = dict(shared)
        m["zT"] = np.ascontiguousarray(f["z"][sl].T)
        m["zTr"] = m["zT"]
        m["cT"] = np.ascontiguousarray(f["c"][sl].T)
        in_maps.append(m)

    nc = _get_nc()
    res = bass_utils.run_bass_kernel_spmd(
        nc, in_maps, list(range(NCORES)), trace=TRACE
    )
    LAST_EXEC_NS = res.exec_time_ns
    LAST_RESULTS = res
    out = np.concatenate(
        [np.asarray(res.results[c]["outT"]).T for c in range(NCORES)], axis=0
    )
    return out
